# revision 19
# baseline (speedup 1.0000x reference)
"""FISTA compressed-sensing kernel for Trainium2 (8 NeuronCores, SPMD).

Problem: for each of 64 patches (x3 channels), run 200 FISTA iterations of
    min_x 0.5||A x - b||^2 + lam||x||_1,   A: (81, 5184)
Sharding: pure data-parallel over the batch — 8 patches x 3 channels = 24
columns per core; A replicated.

Key engineering facts (HW-measured on this container's TRN2s):
  * fp32 matmul at free-dim 24 costs ~300-500 ns/instr; bf16 with full
    128-col weights (FWL) costs ~37-47 ns.  So A is split hi/lo into TWO
    bf16 layers (A = A_hi + A_lo, ~16-bit effective mantissa) and every
    matmul is done twice, accumulating in fp32 PSUM: 164 matmuls/iter at
    ~40 ns beats 82 fp32 matmuls at ~400 ns by ~5x.
  * Custom DVE ops don't compile on this walrus build ("ISA wrong
    length"), so the elementwise tail uses native DVE/Pool/Act ops.
  * Engine instructions cost ~450-500 ns fixed, so elementwise work is
    merged into as few instructions as possible and the FISTA momentum
    coefficients come from a host-precomputed table indexed by the
    hardware loop variable (zero per-iteration coefficient ops).

Numerics (validated against the jax reference in fp64/numpy sim):
  * x/y/d state stays fp32 — bf16 state diverges (momentum amplifies
    state-rounding noise; measured 0.78 rel err).  Only the matmul
    STREAMS are bf16: x cast to bf16 for mm1, Ay recombined to bf16 for
    mm2.  Measured end-to-end rel err ~4.6e-3 (gate 2e-2).
  * -Atb is folded into 24 augmented contraction rows of the mm2
    weights (identity block in the ay rhs), with the bf16 fold split
    hi/lo across the two weight layers => Atb error ~2^-16.
  * Atb itself is computed host-side in fp64.

Per-core per-iteration structure (column matrix X: (5248, 24) fp32):
    d   = X_k - X_{k-1}                        (Pool, 1 op, full size)
    y   = X_k + c_{k-1} d                      (Pool, 1 op, full size)
    U_k = A @ Xbf_k            (82 bf16 matmuls -> fp32 PSUM [128,24])
    du  = U_k - U_{k-1} ; AYbf = U_k + c_{k-1} du   (2 small DVE ops)
    per column group g (psum-bank-sized):
      GP_g = [A;-Atb]^T_hi AYaug + [..]_lo AYaug   (2*ng bf16 matmuls)
      z_g  = y_g - mu*GP_g                     (DVE)
      cl_g = clamp(z_g, +-thr)                 (DVE)
      X_g  = z_g - cl_g                        (DVE, fp32 state)
      Xbf_g = cast(X_g)                        (Act, bf16 mm1 stream)
"""

import os

import numpy as np
import ml_dtypes

import concourse.bass as bass
import concourse.mybir as mybir
import concourse.tile as tile
from concourse.bass import ds
from concourse.bass_utils import run_bass_kernel_spmd

F32 = mybir.dt.float32
BF16 = mybir.dt.bfloat16
NP_BF16 = ml_dtypes.bfloat16

M = 81            # measurements (9x9 camera patch)
D = 5184          # atoms (72x72 upsampled grid)
KT = 41           # 128-row tiles covering D (padded to 5248)
DP = KT * 128     # 5248
NCORES = 8
B = 64
BPC = B // NCORES           # 8 patches per core
N = BPC * 3                 # 24 state columns per core
ITERS = int(os.environ.get("FISTA_ITERS", "200"))
KA = 128                    # augmented contraction dim for matmul2
CT0 = 96                    # partition row where the -Atb^T block starts
GROUPS = [(0, 21), (21, 20)]
UNROLL = 4                  # FISTA iterations per hardware-loop trip

_CACHE = {}


def _legalize_waits(nc):
    """This walrus build accepts at most ONE semaphore wait per instruction
    (setupSyncWait: 'Too many sync wait commands'). Tile emits multi-wait
    instructions; split the excess waits onto injected same-engine NoOps
    placed immediately before the instruction (engine queues are FIFO, so
    semantics are identical)."""
    n = 0
    for fn in nc.m.functions:
        for bb in fn.blocks:
            insts = bb.instructions
            out = []
            changed = False
            for ins in insts:
                si = ins.sync_info
                ow = list(si.on_wait) if si is not None else []
                if len(ow) > 1 and ins.engine is not None:
                    for w in ow[:-1]:
                        n += 1
                        out.append(mybir.InstNoOp(
                            name=f"I-waitnop-{n}",
                            engine=ins.engine,
                            ins=[],
                            outs=[],
                            debug=ins.debug,
                            sync_info=mybir.SyncInfo(on_wait=[w], on_update=[]),
                        ))
                    ins.sync_info = mybir.SyncInfo(
                        on_wait=[ow[-1]], on_update=list(si.on_update))
                    changed = True
                out.append(ins)
            if changed:
                bb.instructions = out
    return n


def _build(mu_s, thr, iters, variant="full"):
    """Build the Bass module (same program for all 8 cores)."""
    do_mm1 = variant in ("full", "mm_only", "mm1_only")
    do_mm2 = variant in ("full", "mm_only", "mm2_only")
    do_ew = variant in ("full", "ew_only")
    nc = bass.Bass()

    athi_d = nc.declare_dram_parameter("athi", [128, KT, 128], BF16,
                                       isOutput=False)
    atlo_d = nc.declare_dram_parameter("atlo", [128, KT, 128], BF16,
                                       isOutput=False)
    w2hi_d = nc.declare_dram_parameter("w2hi", [128, KT, 128], BF16,
                                       isOutput=False)
    w2lo_d = nc.declare_dram_parameter("w2lo", [128, KT, 128], BF16,
                                       isOutput=False)
    ay0_d = nc.declare_dram_parameter("ayinit", [128, N], BF16,
                                      isOutput=False)
    ct_d = nc.declare_dram_parameter("ctab", [128, max(iters, 1)], F32,
                                     isOutput=False)
    xout_d = nc.declare_dram_parameter("xout", [128, KT, N], F32,
                                       isOutput=True)

    with tile.TileContext(nc) as tc:
        with (
            tc.tile_pool(name="weights", bufs=1) as wpool,
            tc.tile_pool(name="state", bufs=1) as spool,
            tc.tile_pool(name="psum_u", bufs=1, space="PSUM") as ppool_u,
            tc.tile_pool(name="psum_gp", bufs=1, space="PSUM") as ppool_gp,
        ):
            # --- persistent SBUF tensors -------------------------------
            at_sb = [wpool.tile([128, KT, 128], BF16, name=f"at{h}")
                     for h in range(2)]
            w2_sb = [wpool.tile([128, KT, 128], BF16, name=f"w2{h}")
                     for h in range(2)]
            ay_sb = wpool.tile([KA, N], BF16)          # rhs for matmul2
            ct_sb = wpool.tile([128, max(iters, 1)], F32)
            y_sb = [spool.tile([128, cnt, N], F32, name=f"y{g}")
                    for g, (off, cnt) in enumerate(GROUPS)]
            x_sb = [[spool.tile([128, cnt, N], F32, name=f"x{s}{g}")
                     for g, (off, cnt) in enumerate(GROUPS)]
                    for s in range(2)]
            xbf_sb = [[spool.tile([128, cnt, N], BF16, name=f"xb{s}{g}")
                       for g, (off, cnt) in enumerate(GROUPS)]
                      for s in range(2)]
            d_sb = [spool.tile([128, cnt, N], F32, name=f"d{g}")
                    for g, (off, cnt) in enumerate(GROUPS)]
            z_sb = [spool.tile([128, cnt, N], F32, name=f"z{g}")
                    for g, (off, cnt) in enumerate(GROUPS)]
            cl_sb = [spool.tile([128, cnt, N], F32, name=f"cl{g}")
                     for g, (off, cnt) in enumerate(GROUPS)]
            u_ps = ppool_u.tile([128, N], F32, tag="u", name="u")
            u_sb = wpool.tile([M, N], F32, name="usb")
            du_sb = wpool.tile([M, N], F32, name="dusb")

            nc.sync.dma_start(out=at_sb[0][:], in_=athi_d[:])
            nc.sync.dma_start(out=at_sb[1][:], in_=atlo_d[:])
            nc.sync.dma_start(out=w2_sb[0][:], in_=w2hi_d[:])
            nc.sync.dma_start(out=w2_sb[1][:], in_=w2lo_d[:])
            nc.sync.dma_start(out=ay_sb[:], in_=ay0_d[:])
            nc.sync.dma_start(out=ct_sb[:], in_=ct_d[:])

            # initial state: x = xbf = 0 (both parities), u_prev = 0
            for g in range(len(GROUPS)):
                nc.vector.memset(x_sb[0][g][:], 0.0)
                nc.vector.memset(x_sb[1][g][:], 0.0)
                nc.vector.memset(xbf_sb[0][g][:], 0.0)
                nc.vector.memset(xbf_sb[1][g][:], 0.0)
            nc.vector.memset(u_sb[:], 0.0)

            kt2g = {}
            for g, (off, cnt) in enumerate(GROUPS):
                for j in range(cnt):
                    kt2g[off + j] = (g, j)

            # --- FISTA iterations: hardware loop, UNROLL per trip ------
            unroll = UNROLL if iters % UNROLL == 0 else 2 if iters % 2 == 0 else 1
            assert iters % unroll == 0
            with tc.For_i(0, iters, unroll,
                          hint_engines=(mybir.EngineType.PE,)) as it:
                for phase in range(unroll):
                    cur, prev = phase % 2, 1 - phase % 2
                    # c_{k-1} for iteration k = it+phase, from the host
                    # table (ctab[:,k] = c_{k-1}); c_{-1} = 0
                    c_prev = ct_sb[:, ds(it + phase, 1)]

                    # d = x_k - x_{k-1} ; y = x_k + c_{k-1} d   (Pool)
                    if do_ew:
                        for g in range(len(GROUPS)):
                            nc.gpsimd.tensor_sub(
                                d_sb[g][:], x_sb[prev][g][:], x_sb[cur][g][:])
                            nc.vector.scalar_tensor_tensor(
                                out=y_sb[g][:],
                                in0=d_sb[g][:],
                                scalar=c_prev,
                                in1=x_sb[prev][g][:],
                                op0=mybir.AluOpType.mult,
                                op1=mybir.AluOpType.add,
                            )

                    # matmul1: U_k = (A_hi + A_lo) @ Xbf_k -> psum [128,24]
                    if do_mm1:
                        for kt in range(KT):
                            g, j = kt2g[kt]
                            for h in range(2):
                                nc.tensor.matmul(
                                    u_ps[:],
                                    at_sb[h][:, kt, :],
                                    xbf_sb[prev][g][:, j, :],
                                    start=(kt == 0 and h == 0),
                                    stop=(kt == KT - 1 and h == 1),
                                )

                    # tiny ay recurrence:
                    # du = U_k - U_{k-1}; AYbf = U_k + c_{k-1} du
                    if do_ew and do_mm1:
                        nc.vector.tensor_sub(
                            du_sb[:], u_ps[0:M, :], u_sb[:])
                        nc.vector.scalar_tensor_tensor(
                            out=ay_sb[0:M, :], in0=du_sb[:],
                            scalar=ct_sb[0:M, ds(it + phase, 1)],
                            in1=u_ps[0:M, :],
                            op0=mybir.AluOpType.mult,
                            op1=mybir.AluOpType.add)
                        # stash U_k for the next iteration (Act engine)
                        nc.scalar.activation(
                            out=u_sb[:], in_=u_ps[0:M, :],
                            func=mybir.ActivationFunctionType.Copy)

                    # matmul2 + soft-threshold tail, per column group
                    for g, (k0, ng) in enumerate(GROUPS):
                        gp = ppool_gp.tile([128, ng, N], F32, tag=f"gp{g}")
                        if do_mm2:
                            for j in range(ng):
                                for h in range(2):
                                    nc.tensor.matmul(
                                        gp[:, j, :],
                                        w2_sb[h][:, k0 + j, :],
                                        ay_sb[:],
                                        start=(h == 0),
                                        stop=(h == 1),
                                    )
                        if not do_ew:
                            continue
                        # z = y - mu*gp ; x = z - clamp(z, +-thr)  (DVE
                        # in-order chain) ; xbf = cast(x) (Act)
                        z = z_sb[g]
                        nc.vector.scalar_tensor_tensor(
                            out=z[:],
                            in0=gp[:] if do_mm2 else y_sb[g][:],
                            scalar=-mu_s,
                            in1=y_sb[g][:],
                            op0=mybir.AluOpType.mult,
                            op1=mybir.AluOpType.add,
                        )
                        cl = cl_sb[g]
                        nc.vector.tensor_scalar(
                            cl[:], z[:], -thr, thr,
                            mybir.AluOpType.max, mybir.AluOpType.min)
                        nc.vector.tensor_sub(x_sb[cur][g][:], z[:], cl[:])
                        nc.scalar.activation(
                            out=xbf_sb[cur][g][:], in_=x_sb[cur][g][:],
                            func=mybir.ActivationFunctionType.Copy)

            # --- write back final x ------------------------------------
            fin = (iters - 1) % 2
            for g, (off, cnt) in enumerate(GROUPS):
                nc.sync.dma_start(out=xout_d[:, off : off + cnt, :],
                                  in_=x_sb[fin][g][:])

    _legalize_waits(nc)
    return nc


def _coef_table(iters):
    """ctab[:, k] = c_{k-1} = (t_{k-1} - 1)/t_k, c_{-1} = 0."""
    cs = np.zeros(max(iters, 1), np.float64)
    t = 1.0
    for k in range(iters - 1):
        tn = (1.0 + np.sqrt(1.0 + 4.0 * t * t)) / 2.0
        cs[k + 1] = (t - 1.0) / tn
        t = tn
    return np.broadcast_to(cs.astype(np.float32), (128, max(iters, 1))).copy()


def _prep_inputs(inp, A, iters=None):
    """Host-side shard/reshape: returns per-core input maps."""
    if iters is None:
        iters = ITERS
    A = np.asarray(A, np.float32)
    A_pad = np.zeros((M, DP), np.float32)
    A_pad[:, :D] = A
    A_hi = A_pad.astype(NP_BF16).astype(np.float32)
    A_lo = (A_pad - A_hi).astype(NP_BF16).astype(np.float32)

    def at_tiles(Ax):
        # [128, KT, 128] bf16: at[p, kt, m] = Ax[m, kt*128+p], cols 81..127
        # zero-padded so FWL (full 128-col weight load) engages
        t = np.zeros((128, KT, 128), np.float32)
        t[:, :, :M] = Ax.T.reshape(KT, 128, M).transpose(1, 0, 2)
        return t.astype(NP_BF16)

    def w2_tiles(Ax):
        # [128, KT, 128] bf16: w2[k, kt, j] = Ax[k, kt*128+j] for k<81;
        # aug rows CT0..CT0+N filled per-core below
        t = np.zeros((128, KT, 128), np.float32)
        t[:M] = Ax.reshape(M, KT, 128)
        return t

    athi = at_tiles(A_hi)
    atlo = at_tiles(A_lo)
    w2hi_base = w2_tiles(A_hi)
    w2lo_base = w2_tiles(A_lo)

    ay_init = np.zeros((128, N), np.float32)
    ay_init[CT0 : CT0 + N] = np.eye(N, dtype=np.float32)
    ay_init = ay_init.astype(NP_BF16)
    ctab = _coef_table(iters)

    inp = np.asarray(inp, np.float32)
    in_maps = []
    for c in range(NCORES):
        chunk = inp[c * BPC : (c + 1) * BPC]            # (8, 81, 3)
        b_mat = chunk.transpose(1, 0, 2).reshape(M, N)  # (81, 24)
        # -Atb in fp64, split hi/lo into the two weight layers' aug rows
        natb = -(A_pad.T.astype(np.float64) @ b_mat.astype(np.float64))
        natb_hi = natb.astype(np.float32).astype(NP_BF16).astype(np.float32)
        natb_lo = (natb - natb_hi).astype(np.float32)
        w2hi = w2hi_base.copy()
        w2lo = w2lo_base.copy()
        # w2[CT0+c, kt, j] = natb[kt*128+j, c]
        w2hi[CT0 : CT0 + N] = natb_hi.reshape(KT, 128, N).transpose(2, 0, 1)
        w2lo[CT0 : CT0 + N] = natb_lo.reshape(KT, 128, N).transpose(2, 0, 1)
        in_maps.append({
            "athi": athi, "atlo": atlo,
            "w2hi": w2hi.astype(NP_BF16), "w2lo": w2lo.astype(NP_BF16),
            "ayinit": ay_init, "ctab": ctab,
        })
    return in_maps


def _unshard(results):
    outs = []
    for c in range(NCORES):
        xo = np.asarray(results[c]["xout"])              # [128, KT, N]
        x_dn = xo.transpose(1, 0, 2).reshape(DP, N)[:D]  # (5184, 24)
        outs.append(x_dn.reshape(72, 72, BPC, 3).transpose(2, 0, 1, 3))
    return np.concatenate(outs, 0).astype(np.float32)    # (64, 72, 72, 3)


def _run(inp, A, lam, mu, trace=False):
    mu_s = float(np.asarray(mu).reshape(-1)[0])
    thr = float(np.asarray(lam).reshape(-1)[0]) * mu_s
    key = (mu_s, thr, ITERS)
    if key not in _CACHE:
        _CACHE[key] = _build(mu_s, thr, ITERS)
    nc = _CACHE[key]
    in_maps = _prep_inputs(inp, A)
    res = run_bass_kernel_spmd(nc, in_maps, list(range(NCORES)), trace=trace)
    return _unshard(res.results), res


def kernel(inp, A, lam, mu):
    out, _ = _run(inp, A, lam, mu)
    return out
